# revision 1
# baseline (speedup 1.0000x reference)
"""Trainium2 Bass kernel for nn_Encoder_block (B=128,S=512,D=24,H=4,HD=6,DFF=48).

Strategy: pure data parallel over batch — 16 batches per NeuronCore x 8 cores.
Per core everything runs in "T-layout" ([d, token] with d on partitions),
processed in 4 groups of 4 batches banded onto the 128 partitions
(batch p of a group occupies partitions 32p..32p+24).

Key tricks:
  - QKV projection emits Q^T/K^T per head at partition bands 32h..32h+6 and
    V^T at rows 8:32 of the same PSUM tile, so scores matmuls can row-tile
    (tile_position=(32h,0)) straight out of one SBUF copy.
  - Scores are computed TRANSPOSED (S^T[k,q]) so softmax's sum over k is a
    partition reduce done for free by an extra ones-column in the AV lhsT.
  - AV is col-tiled 4-heads-per-bank; the softmax denominator rides along as
    lhsT column 6.  Normalization is one reciprocal + gpsimd partition
    broadcasts + one tensor_tensor multiply.
  - LayerNorm runs in T-layout: mean/E[y^2] via band-selector matmuls,
    rstd = exp(-0.5*ln(var+eps)) so the whole kernel uses ONE ACT table set
    (natural_log_exp_and_others) — no table thrash with the softmax exps.
  - Output leaves as 32x32 DVE block-transpose + strided DMA gather.
"""

import os
import sys

import numpy as np

for _p in ("/opt/trn_rl_repo", "/opt/trn_rl_repo/concourse"):
    if os.path.isdir(_p) and _p not in sys.path:
        sys.path.insert(0, _p)

import concourse.bass as bass
import concourse.bacc as bacc
import concourse.mybir as mybir
import concourse.tile as tile
from concourse.bass_utils import run_bass_kernel_spmd

F32 = mybir.dt.float32
BF16 = mybir.dt.bfloat16
BF16_ATTN = True
ADT = BF16 if BF16_ATTN else F32
AF = mybir.ActivationFunctionType
ALU = mybir.AluOpType

B, S, D = 128, 512, 24
H, HD, DFF = 4, 6, 48
EPS = 1e-5
NCORES = 8
NB = B // NCORES          # batches per core = 16
NGROUPS = NB // 4         # groups of 4 banded batches = 4
SCALE = 1.0 / np.sqrt(HD)  # folded into the exp


def _host_consts(Wq, Wk, Wv, Wo, W1, W2, g1, b1, g2, b2):
    """Pre-layout all weights on the host (numpy) into the banded SBUF forms
    the kernel wants.  All float32."""
    c = {}
    # mm1 lhsT (per band replicated): cols 32h+j (j<6) = Wq[6h+j, :],
    # cols 8..32 = Wv rows; result rows are Q^T bands + V^T block.
    wqk1 = np.zeros((D, 128), np.float32)
    wk2 = np.zeros((D, 128), np.float32)
    for h in range(H):
        for j in range(HD):
            wqk1[:, 32 * h + j] = Wq[6 * h + j, :]
            wk2[:, 32 * h + j] = Wk[6 * h + j, :]
    for dv in range(D):
        wqk1[:, 8 + dv] = Wv[dv, :]
    WQK1 = np.zeros((128, 128), np.float32)
    WK2 = np.zeros((128, 128), np.float32)
    for p in range(4):
        WQK1[32 * p : 32 * p + D, :] = wqk1
        WK2[32 * p : 32 * p + D, :] = wk2
    c["wqk1"] = WQK1
    c["wk2"] = WK2

    # Wo lhsT: rows 32h+1+j = Wo[:, 6h+j] (row 32h is the denominator slot)
    # 32 cols so psum rows 24:32 of each output band are written (zeros)
    WOE = np.zeros((128, 32), np.float32)
    for h in range(H):
        for j in range(HD):
            WOE[32 * h + 1 + j, 0:D] = Wo[:, 6 * h + j]
    c["woe"] = WOE

    # band selectors (M=128 so every psum row is written; outputs at rows 32p):
    # cb1 col 32p = -1/24 over band p (gives -mean), cb2 col 32p = +1/24 (E[y^2])
    CB1 = np.zeros((128, 128), np.float32)
    CB2 = np.zeros((128, 128), np.float32)
    for p in range(4):
        CB1[32 * p : 32 * p + D, 32 * p] = -1.0 / D
        CB2[32 * p : 32 * p + D, 32 * p] = 1.0 / D
    c["cb1"] = CB1
    c["cb2"] = CB2

    # FFN W1 lhsT: variant p picks band p: rows 32p+d, col 64p+m = W1[m, d]
    # (64 cols per variant so the full 64-row psum half gets written)
    W1E = np.zeros((128, 4 * 64), np.float32)
    for p in range(4):
        W1E[32 * p : 32 * p + D, 64 * p : 64 * p + DFF] = W1.T
    c["w1e"] = W1E

    # FFN W2 lhsT: even variant rows 0:48, odd variant rows 64:112
    # (32 cols per variant so full psum bands get written)
    W2E = np.zeros((128, 2 * 32), np.float32)
    W2E[0:DFF, 0:D] = W2.T
    W2E[64 : 64 + DFF, 32 : 32 + D] = W2.T
    c["w2e"] = W2E

    # identity for PE transposes
    c["idt"] = np.eye(128, dtype=np.float32)
    if BF16_ATTN:
        import ml_dtypes
        c["idtb"] = np.eye(32, dtype=ml_dtypes.bfloat16)
    else:
        c["idtb"] = np.eye(32, dtype=np.float32)

    # LN gains/biases banded as per-partition scalars [128, 4] = g1,b1,g2,b2
    GB = np.zeros((128, 4), np.float32)
    for p in range(4):
        GB[32 * p : 32 * p + D, 0] = g1
        GB[32 * p : 32 * p + D, 1] = b1
        GB[32 * p : 32 * p + D, 2] = g2
        GB[32 * p : 32 * p + D, 3] = b2
    c["gb"] = GB
    return c


CONST_SHAPES = {
    "wqk1": (128, 128),
    "wk2": (128, 128),
    "woe": (128, 32),
    "cb1": (128, 128),
    "cb2": (128, 128),
    "w1e": (128, 4 * 64),
    "w2e": (128, 2 * 32),
    "idt": (128, 128),
    "idtb": (32, 32),
    "gb": (128, 4),
}


def _pin_act_tables():
    """Force Exp and Ln to resolve to the combined natural_log_exp_and_others
    table set (otherwise the compiler ping-pongs exp_and_others <-> natural_log
    at every LayerNorm, ~1.3us per reload)."""
    import concourse.bacc as _bacc
    if getattr(_bacc, "_act_tables_pinned", False):
        return
    _orig = _bacc.get_activation_tables

    def _patched(arch):
        tables = dict(_orig(arch))
        keep = "natural_log_exp_and_others"
        for name in list(tables):
            if name != keep and (
                AF.Exp in tables[name] or AF.Ln in tables[name]
            ):
                tables[name] = set()
        return tables

    _bacc.get_activation_tables = _patched
    _bacc._act_tables_pinned = True


def build_nc(nb: int = NB) -> bass.Bass:
    """Build the per-core Bass program. nb = batches this core processes."""
    _pin_act_tables()
    ngroups = nb // 4
    nc = bacc.Bacc()
    x_in = nc.dram_tensor("x", [nb, S, D], F32, kind="ExternalInput")
    out = nc.dram_tensor("out", [nb, S, D], F32, kind="ExternalOutput")
    cin = {
        k: nc.dram_tensor(k, list(sh), ADT if k == "idtb" else F32, kind="ExternalInput")
        for k, sh in CONST_SHAPES.items()
    }

    with tile.TileContext(nc) as tc:
        import contextlib

        ctx = contextlib.ExitStack()
        with ctx:
            constp = ctx.enter_context(tc.tile_pool(name="consts", bufs=1))
            xnp = ctx.enter_context(tc.tile_pool(name="xn", bufs=2))
            xtp = ctx.enter_context(tc.tile_pool(name="xt", bufs=2))
            qkp = ctx.enter_context(tc.tile_pool(name="qk", bufs=2))
            vsbp = ctx.enter_context(tc.tile_pool(name="vsb", bufs=2))
            ep = ctx.enter_context(tc.tile_pool(name="e", bufs=2))
            rrp = ctx.enter_context(tc.tile_pool(name="rr", bufs=2))
            rbp = ctx.enter_context(tc.tile_pool(name="rb", bufs=2))
            otp = ctx.enter_context(tc.tile_pool(name="ot", bufs=2))
            y1p = ctx.enter_context(tc.tile_pool(name="y1", bufs=2))
            x1p = ctx.enter_context(tc.tile_pool(name="x1", bufs=2))
            hsp = ctx.enter_context(tc.tile_pool(name="hs", bufs=2))
            fsp = ctx.enter_context(tc.tile_pool(name="fs", bufs=2))
            ysqp = ctx.enter_context(tc.tile_pool(name="ysq", bufs=2))
            bcp = ctx.enter_context(tc.tile_pool(name="bc", bufs=4))
            smp = ctx.enter_context(tc.tile_pool(name="sm", bufs=8))
            ytp = ctx.enter_context(tc.tile_pool(name="yt", bufs=2))
            # PSUM: st(2) + qkv(2) + uo(2) + misc(2) = 8 banks
            stp = ctx.enter_context(tc.tile_pool(name="st", bufs=2, space="PSUM"))
            qkvp = ctx.enter_context(tc.tile_pool(name="qkv", bufs=2, space="PSUM"))
            uop = ctx.enter_context(tc.tile_pool(name="uo", bufs=2, space="PSUM"))
            miscp = ctx.enter_context(tc.tile_pool(name="mp", bufs=2, space="PSUM"))

            # ---- load constants ----
            C = {}
            for k, sh in CONST_SHAPES.items():
                dt = ADT if k == "idtb" else F32
                t = constp.tile(list(sh), dt, name=f"c_{k}")
                nc.sync.dma_start(out=t, in_=cin[k][:, :])
                C[k] = t
            eps_t = constp.tile([128, 1], F32, name="c_eps")
            nc.vector.memset(eps_t, EPS)

            def bcast_rows(dst, src_row):
                """Broadcast one SBUF row to a 32-row band via DMA with a
                step-0 free dim (gpsimd partition_broadcast is broken on HW)."""
                src_b = bass.AP(
                    tensor=src_row.tensor,
                    offset=src_row.offset,
                    ap=[list(src_row.ap[0]), [0, 32]] + [list(x) for x in src_row.ap[1:]],
                )
                nc.sync.dma_start(
                    out=dst.rearrange("p (x q) -> p x q", x=1), in_=src_b
                )

            def ln_block(Y, gcol, bcol, OUT):
                """LayerNorm over d (partition bands) of Y [128,512] in T-layout.
                gcol/bcol: [128,1] per-partition scalar APs. Writes OUT [128,512].
                """
                mps = miscp.tile([128, S], F32, name="mps", tag="mps")
                nc.tensor.matmul(
                    mps[:, :], C["cb1"][:, :], Y[:, :],
                    start=True, stop=True, tile_position=(0, 0),
                )
                MUN = smp.tile([128, S], F32, name="mun", tag="sm")
                nc.vector.tensor_copy(MUN[:, :], mps[:, :])
                YSQ = ysqp.tile([128, S], F32, name="ysq")
                nc.gpsimd.tensor_mul(YSQ[:, :], Y[:, :], Y[:, :])
                m2ps = miscp.tile([128, S], F32, name="m2ps", tag="mps")
                nc.tensor.matmul(
                    m2ps[:, :], C["cb2"][:, :], YSQ[:, :],
                    start=True, stop=True, tile_position=(0, 0),
                )
                MSQ = smp.tile([128, S], F32, name="msq", tag="sm")
                nc.gpsimd.tensor_mul(MSQ[:, :], MUN[:, :], MUN[:, :])
                VAR = smp.tile([128, S], F32, name="var", tag="sm")
                nc.vector.tensor_sub(VAR[:, :], m2ps[:, :], MSQ[:, :])
                LNV = smp.tile([128, S], F32, name="lnv", tag="sm")
                nc.scalar.activation(LNV[:, :], VAR[:, :], AF.Ln, bias=eps_t[:, :])
                RST = smp.tile([128, S], F32, name="rst", tag="sm")
                nc.scalar.activation(RST[:, :], LNV[:, :], AF.Exp, scale=-0.5)
                MUB = bcp.tile([128, S], F32, name="mub", tag="bc")
                RSB = bcp.tile([128, S], F32, name="rsb", tag="bc")
                for p in range(4):
                    bcast_rows(MUB[32 * p : 32 * p + 32, :], MUN[32 * p : 32 * p + 1, :])
                    bcast_rows(RSB[32 * p : 32 * p + 32, :], RST[32 * p : 32 * p + 1, :])
                nc.gpsimd.tensor_add(Y[:, :], Y[:, :], MUB[:, :])  # y - mu
                nc.gpsimd.tensor_mul(Y[:, :], Y[:, :], RSB[:, :])  # * rstd
                nc.vector.tensor_scalar(
                    OUT[:, :], Y[:, :], gcol, bcol, op0=ALU.mult, op1=ALU.add
                )

            for g in range(ngroups):
                XT4 = xtp.tile([128, S], F32, name="xt4")
                Y1 = y1p.tile([128, S], F32, name="y1")
                # x load: block layout U[32p+c, 32f+d] = x[b, 32f+c, d], then a
                # single 32x32 block transpose turns it into XT4 bands.
                U = xnp.tile([128, 16, 32], F32, name="xu")
                nc.vector.memset(U[:, :, D:32], 0.0)
                for p in range(4):
                    b = 4 * g + p
                    nc.sync.dma_start(
                        out=U[32 * p : 32 * p + 32, :, 0:D],
                        in_=x_in[b].rearrange("(f c) d -> c f d", c=32),
                    )
                nc.vector.transpose(XT4[:, :], U.rearrange("p a b -> p (a b)"))
                # ---------- per-batch attention ----------
                for p in range(4):
                    b = 4 * g + p
                    # QKV projections (row-tiled at band p)
                    QK = qkp.tile([128, 2 * S], ADT, name="qk")
                    ps1 = qkvp.tile([128, S], F32, name="ps1", tag="qkv")
                    nc.tensor.matmul(
                        ps1[:, :],
                        C["wqk1"][32 * p : 32 * p + D, :],
                        XT4[32 * p : 32 * p + D, :],
                        start=True, stop=True, tile_position=(32 * p, 0),
                    )
                    nc.vector.tensor_copy(QK[:, 0:S], ps1[:, :])
                    ps2 = qkvp.tile([128, S], F32, name="ps2", tag="qkv")
                    nc.tensor.matmul(
                        ps2[:, :],
                        C["wk2"][32 * p : 32 * p + D, :],
                        XT4[32 * p : 32 * p + D, :],
                        start=True, stop=True, tile_position=(32 * p, 0),
                    )
                    nc.vector.tensor_copy(QK[:, S : 2 * S], ps2[:, :])

                    # V: PE-transpose rows 0:32 of QK (V^T lives at rows 8:32)
                    psv = miscp.tile([128, 4 * 32], ADT, name="psv", tag="mps")
                    for t in range(4):
                        nc.tensor.transpose(
                            psv[:, 32 * t : 32 * (t + 1)],
                            QK[0:32, 128 * t : 128 * (t + 1)],
                            C["idtb"][:, :],
                        )
                    Vsb = vsbp.tile([128, 4, 4, 32], ADT, name="vsb")
                    nc.vector.memset(Vsb[:, :, :, :], 1.0)
                    nc.vector.tensor_copy(
                        Vsb[:, :, :, 1:7],
                        psv.rearrange("p (t x) -> p t x", t=4)[:, :, 8:32].rearrange(
                            "p t (h d) -> p t h d", d=6
                        ),
                    )

                    # scores (S^T) + exp, head-pairs share a 2-bank psum tile
                    E = ep.tile([128, 4, 4, S], ADT, name="e")
                    for t in range(4):
                        for h in range(4):
                            stt = stp.tile([128, S], F32, name="stt", tag="st")
                            nc.tensor.matmul(
                                stt[:, :],
                                QK[32 * h : 32 * h + HD, S + 128 * t : S + 128 * (t + 1)],
                                QK[32 * h : 32 * h + HD, 0:S],
                                start=True, stop=True,
                                tile_position=(32 * h, 0),
                            )
                            nc.scalar.activation(
                                E[:, t, h, :],
                                stt[:, :],
                                AF.Exp,
                                scale=float(SCALE),
                            )

                    # AV col-tiled by head; ones column -> denominators
                    UO = uop.tile([128, S], F32, name="uo")
                    for t in range(4):
                        for h in range(4):
                            nc.tensor.matmul(
                                UO[32 * h : 32 * h + 32, :],
                                Vsb[:, t, h, :],
                                E[:, t, h, :],
                                start=(t == 0), stop=(t == 3),
                                tile_position=(0, 32 * h),
                                skip_group_check=True,
                            )
                    RR = rrp.tile([128, S], F32, name="rrt")
                    nc.vector.reciprocal_approx_fast(RR[:, :], UO[:, :])
                    RB = rbp.tile([128, S], F32, name="rbt")
                    for h in range(4):
                        bcast_rows(RB[32 * h : 32 * h + 32, :], RR[32 * h : 32 * h + 1, :])
                    OTn = otp.tile([128, S], F32, name="otn")
                    nc.vector.tensor_mul(OTn[:, :], UO[:, :], RB[:, :])

                    # Wo projection -> band p of wo psum, then residual into Y1
                    wops = miscp.tile([32, S], F32, name="wops", tag="mps")
                    nc.tensor.matmul(
                        wops[:, :], C["woe"][:, :], OTn[:, :],
                        start=True, stop=True, tile_position=(0, 0),
                    )
                    nc.vector.tensor_add(
                        Y1[32 * p : 32 * p + 32, :],
                        wops[:, :],
                        XT4[32 * p : 32 * p + 32, :],
                    )

                # ---------- LN1 ----------
                X1 = x1p.tile([128, S], F32, name="x1")
                ln_block(Y1, C["gb"][:, 0:1], C["gb"][:, 1:2], X1)

                # ---------- FFN ----------
                F4s = None
                FS = fsp.tile([128, S], F32, name="fs")
                F4 = miscp.tile([128, S], F32, name="f4", tag="mps")
                for pair in range(2):
                    hps = miscp.tile([128, S], F32, name="hps", tag="mps")
                    for j in range(2):
                        p = 2 * pair + j
                        nc.tensor.matmul(
                            hps[64 * j : 64 * j + 64, :],
                            C["w1e"][:, 64 * p : 64 * (p + 1)],
                            X1[:, :],
                            start=True, stop=True, tile_position=(0, 64 * j),
                        )
                    HS = hsp.tile([128, S], F32, name="hs")
                    nc.vector.tensor_scalar_max(HS[:, :], hps[:, :], 0.0)
                    for j in range(2):
                        p = 2 * pair + j
                        nc.tensor.matmul(
                            F4[32 * p : 32 * p + 32, :],
                            C["w2e"][:, 32 * j : 32 * (j + 1)],
                            HS[:, :],
                            start=True, stop=True, tile_position=(0, 32 * p),
                            skip_group_check=True,
                        )
                nc.vector.tensor_scalar_max(FS[:, :], F4[:, :], 0.0)  # 2nd relu
                nc.gpsimd.tensor_add(FS[:, :], FS[:, :], X1[:, :])  # + x1

                # ---------- LN2 + output ----------
                Y2N = x1p.tile([128, S], F32, name="y2n", tag="x1b")
                ln_block(FS, C["gb"][:, 2:3], C["gb"][:, 3:4], Y2N)
                Y2T = ytp.tile([128, S], F32, name="y2t")
                nc.vector.transpose(Y2T[:, :], Y2N[:, :])
                for p in range(4):
                    b = 4 * g + p
                    nc.sync.dma_start(
                        out=out[b].rearrange("(f r) d -> r f d", r=32),
                        in_=Y2T[32 * p : 32 * p + 32, :].rearrange(
                            "r (f c) -> r f c", c=32
                        )[:, :, 0:D],
                    )
    nc.compile()
    return nc


_NC_CACHE: dict[int, bass.Bass] = {}


def _get_nc(nb: int) -> bass.Bass:
    if nb not in _NC_CACHE:
        _NC_CACHE[nb] = build_nc(nb)
    return _NC_CACHE[nb]


def kernel(x, Wq, Wk, Wv, Wo, W1, W2, g1, b1, g2, b2):
    x = np.asarray(x, np.float32)
    consts = _host_consts(
        *(np.asarray(a, np.float32) for a in (Wq, Wk, Wv, Wo, W1, W2, g1, b1, g2, b2))
    )
    nc = _get_nc(NB)
    in_maps = []
    for c in range(NCORES):
        m = {"x": np.ascontiguousarray(x[c * NB : (c + 1) * NB])}
        m.update(consts)
        in_maps.append(m)
    res = run_bass_kernel_spmd(nc, in_maps, list(range(NCORES)))
    return np.concatenate([r["out"] for r in res.results], axis=0)



# revision 4
# speedup vs baseline: 1.5228x; 1.5228x over previous
"""Trainium2 Bass kernel for nn_Encoder_block (B=128,S=512,D=24,H=4,HD=6,DFF=48).

Data parallel over batch: 16 batches/core x 8 cores. Per core, T-layout
([d, token], d on partitions) with 4 batches banded per 128 partitions.

v2 speedups over the 478us baseline:
  - bf16 QKV/Wo matmuls and fp32r LN/FFN matmuls (1 cyc/row instead of 4).
  - softmax exp is a single Schraudolph-style affine per score pair: fp8e4m3
    BITS = round(s_raw * 8*log2e/sqrt(6) + 55.66) computed by one
    tensor_scalar/activation into a uint8-bitcast of the E tile. Replaces
    exact ACT exp + separate fp8 quantize.
  - AV uses fp8 DoubleRow matmuls whose two planes carry two HEADS via
    block-diagonal V weights: 8 matmuls x 512 cols x 0.5 cyc covers the whole
    attention-value product, landing directly in banded T-layout with the
    softmax denominators riding along as ones-columns.
  - softmax normalize: ACT copy UO->bf16, PE selector-broadcast of the
    denominator row, DVE reciprocal + multiply (no DMA broadcasts).
  - LayerNorm: selector matmuls for mean/var, PE broadcast of -mu and
    g*rstd (g folded into the selector weights), everything fp32r.
  - per-batch "tail" (normalize+Wo) emitted one batch late so PE stays busy.
"""

import os
import sys

import numpy as np

for _p in ("/opt/trn_rl_repo", "/opt/trn_rl_repo/concourse"):
    if os.path.isdir(_p) and _p not in sys.path:
        sys.path.insert(0, _p)

import concourse.bass as bass
import concourse.bacc as bacc
import concourse.mybir as mybir
import concourse.tile as tile
from concourse.bass_utils import run_bass_kernel_spmd

F32 = mybir.dt.float32
F32R = mybir.dt.float32r
BF16 = mybir.dt.bfloat16
FP8 = mybir.dt.float8e4
U8 = mybir.dt.uint8
AF = mybir.ActivationFunctionType
ALU = mybir.AluOpType
DR = mybir.MatmulPerfMode.DoubleRow

B, S, D = 128, 512, 24
H, HD, DFF = 4, 6, 48
EPS = 1e-5
NCORES = 8
NB = B // NCORES          # batches per core = 16
SCALE = 1.0 / np.sqrt(HD)
EA = float(8.0 * np.log2(np.e) * SCALE)   # fp8e4m3 bits slope
EB = 55.66                                # fp8e4m3 bits offset (calibrated)

# E-gen engine per (t, hp) slot: balance ACT vs DVE load
EGEN_ENGINES = ["act", "dve", "act", "dve", "act", "dve", "act", "act"]


def _host_consts(Wq, Wk, Wv, Wo, W1, W2, g1, b1, g2, b2):
    import ml_dtypes
    c = {}
    # QKV lhsT (bf16): per band p: col 32h+j = Wq[6h+j,:], cols 8:32 = Wv rows
    wqk1 = np.zeros((D, 128), np.float32)
    wk2 = np.zeros((D, 128), np.float32)
    for h in range(H):
        for j in range(HD):
            wqk1[:, 32 * h + j] = Wq[6 * h + j, :]
            wk2[:, 32 * h + j] = Wk[6 * h + j, :]
    for dv in range(D):
        wqk1[:, 8 + dv] = Wv[dv, :]
    WQK1 = np.zeros((128, 128), np.float32)
    WK2 = np.zeros((128, 128), np.float32)
    for p in range(4):
        WQK1[32 * p : 32 * p + D, :] = wqk1
        WK2[32 * p : 32 * p + D, :] = wk2
    c["wqk1"] = WQK1.astype(ml_dtypes.bfloat16)
    c["wk2"] = WK2.astype(ml_dtypes.bfloat16)

    # Wo lhsT bf16: rows 32h+1+j = Wo[:, 6h+j] (row 32h is the denominator)
    WOE = np.zeros((128, 32), np.float32)
    for h in range(H):
        for j in range(HD):
            WOE[32 * h + 1 + j, 0:D] = Wo[:, 6 * h + j]
    c["woe"] = WOE.astype(ml_dtypes.bfloat16)

    # LN selectors (fp32r): cb1 col 32p = -1/24 over band p; cb2 = +1/24
    CB1 = np.zeros((128, 128), np.float32)
    CB2 = np.zeros((128, 128), np.float32)
    for p in range(4):
        CB1[32 * p : 32 * p + D, 32 * p] = -1.0 / D
        CB2[32 * p : 32 * p + D, 32 * p] = 1.0 / D
    c["cb1"] = CB1
    c["cb2"] = CB2

    # broadcast selectors: col m -> 1 at row 32*(m//32); selg folds g
    SELR = np.zeros((128, 128), np.float32)
    SELG1 = np.zeros((128, 128), np.float32)
    SELG2 = np.zeros((128, 128), np.float32)
    for m in range(128):
        SELR[32 * (m // 32), m] = 1.0
        if m % 32 < D:
            SELG1[32 * (m // 32), m] = g1[m % 32]
            SELG2[32 * (m // 32), m] = g2[m % 32]
    c["selr"] = SELR
    c["selg1"] = SELG1
    c["selg2"] = SELG2
    c["selb"] = SELR.astype(ml_dtypes.bfloat16)

    # FFN W1 lhsT fp32r: variant p: rows 32p+d, col 64p+m = W1[m, d]
    W1E = np.zeros((128, 4 * 64), np.float32)
    for p in range(4):
        W1E[32 * p : 32 * p + D, 64 * p : 64 * p + DFF] = W1.T
    c["w1e"] = W1E.astype(ml_dtypes.bfloat16)

    # FFN W2 lhsT bf16: even variant rows 0:48, odd rows 64:112
    W2E = np.zeros((128, 2 * 32), np.float32)
    W2E[0:DFF, 0:D] = W2.T
    W2E[64 : 64 + DFF, 32 : 32 + D] = W2.T
    c["w2e"] = W2E.astype(ml_dtypes.bfloat16)

    c["idtb"] = np.eye(32, dtype=ml_dtypes.bfloat16)

    # banded biases (only used when nonzero)
    GB = np.zeros((128, 2), np.float32)
    for p in range(4):
        GB[32 * p : 32 * p + D, 0] = b1
        GB[32 * p : 32 * p + D, 1] = b2
    c["gb"] = GB
    return c


CONST_SPECS = {
    "wqk1": ((128, 128), BF16),
    "wk2": ((128, 128), BF16),
    "woe": ((128, 32), BF16),
    "cb1": ((128, 128), F32R),
    "cb2": ((128, 128), F32R),
    "selr": ((128, 128), F32R),
    "selg1": ((128, 128), F32R),
    "selg2": ((128, 128), F32R),
    "selb": ((128, 128), BF16),
    "w1e": ((128, 4 * 64), BF16),
    "w2e": ((128, 2 * 32), BF16),
    "idtb": ((32, 32), BF16),
    "gb": ((128, 2), F32),
}


def _pin_act_tables():
    """Pin Exp/Ln to natural_log_exp_and_others so LN's Ln+Exp never thrash."""
    import concourse.bacc as _bacc
    if getattr(_bacc, "_act_tables_pinned", False):
        return
    _orig = _bacc.get_activation_tables

    def _patched(arch):
        tables = dict(_orig(arch))
        keep = "natural_log_exp_and_others"
        for name in list(tables):
            if name != keep and (AF.Exp in tables[name] or AF.Ln in tables[name]):
                tables[name] = set()
        return tables

    _bacc.get_activation_tables = _patched
    _bacc._act_tables_pinned = True


def build_nc(nb: int = NB, use_b: bool = False) -> bass.Bass:
    _pin_act_tables()
    ngroups = nb // 4
    nc = bacc.Bacc()
    x_in = nc.dram_tensor("x", [nb, S, D], F32, kind="ExternalInput")
    out = nc.dram_tensor("out", [nb, S, D], F32, kind="ExternalOutput")
    cin = {
        k: nc.dram_tensor(k, list(sh), dt, kind="ExternalInput")
        for k, (sh, dt) in CONST_SPECS.items()
    }

    with tile.TileContext(nc) as tc:
        import contextlib

        ctx = contextlib.ExitStack()
        with ctx:
            constp = ctx.enter_context(tc.tile_pool(name="consts", bufs=1))
            vdp = ctx.enter_context(tc.tile_pool(name="vdp", bufs=1))
            xnp = ctx.enter_context(tc.tile_pool(name="xn", bufs=2))
            xtp = ctx.enter_context(tc.tile_pool(name="xt", bufs=2))
            xtbp = ctx.enter_context(tc.tile_pool(name="xtb", bufs=2))
            qkp = ctx.enter_context(tc.tile_pool(name="qk", bufs=2))
            ep = ctx.enter_context(tc.tile_pool(name="e", bufs=2))
            uosp = ctx.enter_context(tc.tile_pool(name="uos", bufs=2))
            rcpp = ctx.enter_context(tc.tile_pool(name="rcp", bufs=2))
            otp = ctx.enter_context(tc.tile_pool(name="ot", bufs=2))
            y1p = ctx.enter_context(tc.tile_pool(name="y1", bufs=2))
            lnp = ctx.enter_context(tc.tile_pool(name="ln", bufs=2))
            x1p = ctx.enter_context(tc.tile_pool(name="x1", bufs=2))
            hsp = ctx.enter_context(tc.tile_pool(name="hs", bufs=2))
            fsp = ctx.enter_context(tc.tile_pool(name="fs", bufs=2))
            ytp = ctx.enter_context(tc.tile_pool(name="yt", bufs=2))
            # PSUM: stt 2x[128,1024] + half 2x[128,512] + uo 2x[128,512] = 8 banks
            psA = ctx.enter_context(tc.tile_pool(name="psA", bufs=2, space="PSUM"))
            psB = ctx.enter_context(tc.tile_pool(name="psB", bufs=2, space="PSUM"))
            psC = ctx.enter_context(tc.tile_pool(name="psC", bufs=2, space="PSUM"))

            C = {}
            for k, (sh, dt) in CONST_SPECS.items():
                t = constp.tile(list(sh), dt, name=f"c_{k}")
                nc.sync.dma_start(out=t, in_=cin[k][:, :])
                C[k] = t
            eps_t = constp.tile([128, 1], F32, name="c_eps")
            nc.vector.memset(eps_t, EPS)

            # two persistent VD tiles [128, t4, hp2, i2, m128] fp8 (ping-pong).
            # ones preset at m = 32*(2hp+i) + {0, 7..31} inside each head band
            # (denominator column + finite padding rows for UO).
            vd_tiles = []
            for v in range(2):
                vt = vdp.tile([128, 4, 2, 2, 128], FP8, name=f"vd{v}")
                nc.gpsimd.memset(vt[:, :, :, :, :], 0.0)
                for hp in range(2):
                    for i in range(2):
                        m0 = 32 * (2 * hp + i)
                        nc.gpsimd.memset(vt[:, :, hp, i, m0 : m0 + 1], 1.0)
                        nc.gpsimd.memset(vt[:, :, hp, i, m0 + 7 : m0 + 32], 1.0)
                vd_tiles.append(vt)

            def ln_block(Y, selg, bslice, OUT):
                """LayerNorm over banded d of Y [128,512] fp32r. OUT = LN(Y)."""
                mps = psB.tile([128, S], F32, name="mps", tag="half")
                nc.tensor.matmul(mps[:, :], C["cb1"][:, :], Y[:, :],
                                 start=True, stop=True)
                MU1 = lnp.tile([128, S], F32R, name="mu1", tag="mu")
                nc.vector.tensor_copy(MU1[:, :], mps[:, :])
                bcmu = psB.tile([128, S], F32, name="bcmu", tag="half")
                nc.tensor.matmul(bcmu[:, :], C["selr"][:, :], MU1[:, :],
                                 start=True, stop=True)
                YC = lnp.tile([128, S], F32R, name="yc", tag="yc")
                nc.vector.tensor_add(YC[:, :], Y[:, :], bcmu[:, :])
                YC2 = lnp.tile([128, S], F32R, name="yc2", tag="yc2")
                nc.gpsimd.tensor_mul(YC2[:, :], YC[:, :], YC[:, :])
                vps = psB.tile([128, S], F32, name="vps", tag="half")
                nc.tensor.matmul(vps[:, :], C["cb2"][:, :], YC2[:, :],
                                 start=True, stop=True)
                LNV = lnp.tile([128, S], F32, name="lnv", tag="lnv")
                nc.scalar.activation(LNV[:, :], vps[:, :], AF.Ln, bias=eps_t[:, :])
                RS1 = lnp.tile([128, S], F32R, name="rs1", tag="rs")
                nc.scalar.activation(RS1[:, :], LNV[:, :], AF.Exp, scale=-0.5)
                bcrs = psB.tile([128, S], F32, name="bcrs", tag="half")
                nc.tensor.matmul(bcrs[:, :], selg[:, :], RS1[:, :],
                                 start=True, stop=True)
                if use_b:
                    TMP = lnp.tile([128, S], F32R, name="lntmp", tag="tmp")
                    nc.vector.tensor_mul(TMP[:, :], YC[:, :], bcrs[:, :])
                    nc.vector.tensor_scalar(out=OUT[:, :], in0=TMP[:, :],
                                            scalar1=bslice, scalar2=None,
                                            op0=ALU.add)
                else:
                    nc.vector.tensor_mul(OUT[:, :], YC[:, :], bcrs[:, :])

            def batch_head(g, p, XT4b):
                """QKV + scores + E-gen + AV for batch 4g+p. Returns state."""
                ps1 = psB.tile([128, S], F32, name="ps1", tag="half")
                nc.tensor.matmul(
                    ps1[:, :], C["wqk1"][32 * p : 32 * p + D, :],
                    XT4b[32 * p : 32 * p + D, :],
                    start=True, stop=True, tile_position=(32 * p, 0),
                )
                ps2 = psB.tile([128, S], F32, name="ps2", tag="half")
                nc.tensor.matmul(
                    ps2[:, :], C["wk2"][32 * p : 32 * p + D, :],
                    XT4b[32 * p : 32 * p + D, :],
                    start=True, stop=True, tile_position=(32 * p, 0),
                )
                QK = qkp.tile([128, 2 * S], BF16, name="qk")
                nc.vector.tensor_copy(QK[:, 0:S], ps1[:, :])
                nc.scalar.activation(QK[:, S : 2 * S], ps2[:, :], AF.Copy)

                # V^T via PE transposes of rows 0:32 (V lives at rows 8:32)
                psv = psB.tile([128, 4 * 32], BF16, name="psv", tag="half")
                for t in range(4):
                    nc.tensor.transpose(
                        psv[:, 32 * t : 32 * (t + 1)],
                        QK[0:32, 128 * t : 128 * (t + 1)],
                        C["idtb"][:, :],
                    )
                VD = vd_tiles[(4 * g + p) % 2]
                # VD[k, t, hp, i, 32*(2hp+i)+1 : +7] = psv[k, 32t+8+6(2hp+i) : +6]
                vd_dst = bass.AP(
                    tensor=VD.tensor, offset=VD.offset + 1,
                    ap=[list(VD.ap[0]), [512, 4], [320, 2], [160, 2], [1, 6]],
                )
                vd_src = bass.AP(
                    tensor=psv.tensor, offset=psv.offset + 8,
                    ap=[list(psv.ap[0]), [32, 4], [12, 2], [6, 2], [1, 6]],
                )
                nc.vector.tensor_copy(vd_dst, vd_src)

                # scores (S^T, raw) + E-gen (fp8e4m3 bits via affine)
                E = ep.tile([128, 4, 2, 2, S], FP8, name="e")
                for t in range(4):
                    for hp in range(2):
                        stt = psA.tile([128, 2 * S], F32, name="stt", tag="big")
                        for i in range(2):
                            h = 2 * hp + i
                            nc.tensor.matmul(
                                stt[:, 512 * i : 512 * (i + 1)],
                                QK[32 * h : 32 * h + HD,
                                   S + 128 * t : S + 128 * (t + 1)],
                                QK[32 * h : 32 * h + HD, 0:S],
                                start=True, stop=True,
                                tile_position=(32 * h, 0),
                                skip_group_check=True,
                            )
                        eng = EGEN_ENGINES[2 * t + hp]
                        e_out = E[:, t, hp, :, :].bitcast(U8)
                        s_in = stt.rearrange("p (i q) -> p i q", i=2)
                        if eng == "act":
                            nc.scalar.activation(e_out, s_in, AF.Copy,
                                                 bias=EB, scale=EA)
                        else:
                            nc.vector.tensor_scalar(
                                out=e_out, in0=s_in, scalar1=EA, scalar2=EB,
                                op0=ALU.mult, op1=ALU.add,
                            )

                # AV: fp8 DoubleRow, two heads per matmul via block-diag VD
                UO = psC.tile([128, S], F32, name="uo", tag="uo")
                idx = 0
                for t in range(4):
                    for hp in range(2):
                        nc.tensor.matmul(
                            UO[:, :], VD[:, t, hp, :, :], E[:, t, hp, :, :],
                            start=(idx == 0), stop=(idx == 7),
                            perf_mode=DR, tile_position=(0, 0),
                        )
                        idx += 1
                return {"UO": UO, "p": p}

            def batch_tail(state, XT4, Y1):
                """normalize + Wo + residual for a batch (lagged emission)."""
                UO = state["UO"]
                p = state["p"]
                UOs = uosp.tile([128, S], BF16, name="uos")
                nc.scalar.activation(UOs[:, :], UO[:, :], AF.Copy)
                denps = psB.tile([128, S], F32, name="denps", tag="half")
                nc.tensor.matmul(denps[:, :], C["selb"][:, :], UOs[:, :],
                                 start=True, stop=True)
                RCP = rcpp.tile([128, S], F32, name="rcpt")
                nc.vector.reciprocal_approx_fast(RCP[:, :], denps[:, :])
                OTn = otp.tile([128, S], BF16, name="otn")
                nc.vector.tensor_mul(OTn[:, :], UOs[:, :], RCP[:, :])
                wops = psB.tile([32, S], F32, name="wops", tag="half")
                nc.tensor.matmul(wops[:, :], C["woe"][:, :], OTn[:, :],
                                 start=True, stop=True)
                nc.vector.tensor_add(
                    Y1[32 * p : 32 * p + 32, :], wops[:, :],
                    XT4[32 * p : 32 * p + 32, :],
                )

            def do_group(g, XT4, Y1):
                """LN1 + FFN + LN2 + output store for group g."""
                X1 = x1p.tile([128, S], F32R, name="x1")
                ln_block(Y1, C["selg1"], C["gb"][:, 0:1], X1)
                X1b = x1p.tile([128, S], BF16, name="x1b", tag="x1b")
                nc.gpsimd.tensor_copy(X1b[:, :], X1[:, :])

                F4 = psC.tile([128, S], F32, name="f4", tag="uo")
                for pair in range(2):
                    hps = psB.tile([128, S], F32, name="hps", tag="half")
                    for j in range(2):
                        p = 2 * pair + j
                        nc.tensor.matmul(
                            hps[64 * j : 64 * j + 64, :],
                            C["w1e"][:, 64 * p : 64 * (p + 1)],
                            X1b[:, :],
                            start=True, stop=True, tile_position=(0, 64 * j),
                            skip_group_check=True,
                        )
                    HS = hsp.tile([128, S], BF16, name="hs")
                    nc.scalar.activation(HS[:, :], hps[:, :], AF.Relu)
                    for j in range(2):
                        p = 2 * pair + j
                        nc.tensor.matmul(
                            F4[32 * p : 32 * p + 32, :],
                            C["w2e"][:, 32 * j : 32 * (j + 1)],
                            HS[:, :],
                            start=True, stop=True, tile_position=(0, 32 * p),
                            skip_group_check=True,
                        )
                FS0 = fsp.tile([128, S], F32R, name="fs0", tag="fs0")
                nc.scalar.activation(FS0[:, :], F4[:, :], AF.Relu)
                FS = fsp.tile([128, S], F32R, name="fst", tag="fst")
                nc.gpsimd.tensor_add(FS[:, :], FS0[:, :], X1[:, :])

                Y2N = ytp.tile([128, S], F32, name="y2n", tag="y2n")
                ln_block(FS, C["selg2"], C["gb"][:, 1:2], Y2N)
                Y2T = ytp.tile([128, S], F32, name="y2t", tag="y2t")
                nc.vector.transpose(Y2T[:, :], Y2N[:, :])
                for p in range(4):
                    b = 4 * g + p
                    nc.sync.dma_start(
                        out=out[b].rearrange("(f r) d -> r f d", r=32),
                        in_=Y2T[32 * p : 32 * p + 32, :].rearrange(
                            "r (f c) -> r f c", c=32
                        )[:, :, 0:D],
                    )

            prev = None
            for g in range(ngroups):
                U = xnp.tile([128, 16, 32], F32, name="xu")
                nc.gpsimd.memset(U[:, :, D:32], 0.0)
                for p in range(4):
                    b = 4 * g + p
                    nc.sync.dma_start(
                        out=U[32 * p : 32 * p + 32, :, 0:D],
                        in_=x_in[b].rearrange("(f c) d -> c f d", c=32),
                    )
                XT4 = xtp.tile([128, S], F32, name="xt4")
                nc.vector.transpose(XT4[:, :], U.rearrange("p a b -> p (a b)"))
                XT4b = xtbp.tile([128, S], BF16, name="xt4b")
                nc.gpsimd.tensor_copy(XT4b[:, :], XT4[:, :])
                Y1 = y1p.tile([128, S], F32R, name="y1")

                for p in range(4):
                    state = batch_head(g, p, XT4b)
                    if prev is not None:
                        batch_tail(*prev)
                    prev = (state, XT4, Y1)
                batch_tail(*prev)
                prev = None
                do_group(g, XT4, Y1)

    nc.compile()
    return nc


_NC_CACHE: dict[tuple, bass.Bass] = {}


def _get_nc(nb: int, use_b: bool = False) -> bass.Bass:
    key = (nb, use_b)
    if key not in _NC_CACHE:
        _NC_CACHE[key] = build_nc(nb, use_b)
    return _NC_CACHE[key]


def kernel(x, Wq, Wk, Wv, Wo, W1, W2, g1, b1, g2, b2):
    x = np.asarray(x, np.float32)
    args = [np.asarray(a, np.float32) for a in (Wq, Wk, Wv, Wo, W1, W2, g1, b1, g2, b2)]
    consts = _host_consts(*args)
    use_b = bool(np.any(args[7]) or np.any(args[9]))
    nc = _get_nc(NB, use_b)
    in_maps = []
    for c in range(NCORES):
        m = {"x": np.ascontiguousarray(x[c * NB : (c + 1) * NB])}
        m.update(consts)
        in_maps.append(m)
    res = run_bass_kernel_spmd(nc, in_maps, list(range(NCORES)))
    return np.concatenate([r["out"] for r in res.results], axis=0)


# revision 5
# speedup vs baseline: 1.8591x; 1.2208x over previous
"""Trainium2 Bass kernel for nn_Encoder_block (B=128,S=512,D=24,H=4,HD=6,DFF=48).

Data parallel over batch: 16 batches/core x 8 cores. Per core, T-layout
([d, token], d on partitions) with 4 batches banded per 128 partitions.

v2 speedups over the 478us baseline:
  - bf16 QKV/Wo matmuls and fp32r LN/FFN matmuls (1 cyc/row instead of 4).
  - softmax exp is a single Schraudolph-style affine per score pair: fp8e4m3
    BITS = round(s_raw * 8*log2e/sqrt(6) + 55.66) computed by one
    tensor_scalar/activation into a uint8-bitcast of the E tile. Replaces
    exact ACT exp + separate fp8 quantize.
  - AV uses fp8 DoubleRow matmuls whose two planes carry two HEADS via
    block-diagonal V weights: 8 matmuls x 512 cols x 0.5 cyc covers the whole
    attention-value product, landing directly in banded T-layout with the
    softmax denominators riding along as ones-columns.
  - softmax normalize: ACT copy UO->bf16, PE selector-broadcast of the
    denominator row, DVE reciprocal + multiply (no DMA broadcasts).
  - LayerNorm: selector matmuls for mean/var, PE broadcast of -mu and
    g*rstd (g folded into the selector weights), everything fp32r.
  - per-batch "tail" (normalize+Wo) emitted one batch late so PE stays busy.
"""

import os
import sys

import numpy as np

for _p in ("/opt/trn_rl_repo", "/opt/trn_rl_repo/concourse"):
    if os.path.isdir(_p) and _p not in sys.path:
        sys.path.insert(0, _p)

import concourse.bass as bass
import concourse.bacc as bacc
import concourse.mybir as mybir
import concourse.tile as tile
from concourse.bass_utils import run_bass_kernel_spmd

F32 = mybir.dt.float32
F32R = mybir.dt.float32r
BF16 = mybir.dt.bfloat16
FP8 = mybir.dt.float8e4
U8 = mybir.dt.uint8
AF = mybir.ActivationFunctionType
ALU = mybir.AluOpType
DR = mybir.MatmulPerfMode.DoubleRow

B, S, D = 128, 512, 24
H, HD, DFF = 4, 6, 48
EPS = 1e-5
NCORES = 8
NB = B // NCORES          # batches per core = 16
SCALE = 1.0 / np.sqrt(HD)
EA = float(8.0 * np.log2(np.e) * SCALE)   # fp8e4m3 bits slope
EB = 55.66                                # fp8e4m3 bits offset (calibrated)

# E-gen engine per (t, hp) slot: balance ACT vs DVE load
EGEN_ENGINES = ["act", "dve", "act", "dve", "act", "dve", "act", "act"]


def _host_consts(Wq, Wk, Wv, Wo, W1, W2, g1, b1, g2, b2):
    import ml_dtypes
    c = {}
    # QKV lhsT (bf16): per band p: col 32h+j = Wq[6h+j,:], cols 8:32 = Wv rows
    wqk1 = np.zeros((D, 128), np.float32)
    wk2 = np.zeros((D, 128), np.float32)
    for h in range(H):
        for j in range(HD):
            wqk1[:, 32 * h + j] = Wq[6 * h + j, :]
            wk2[:, 32 * h + j] = Wk[6 * h + j, :]
    for dv in range(D):
        wqk1[:, 8 + dv] = Wv[dv, :]
    WQK1 = np.zeros((128, 128), np.float32)
    WK2 = np.zeros((128, 128), np.float32)
    for p in range(4):
        WQK1[32 * p : 32 * p + D, :] = wqk1
        WK2[32 * p : 32 * p + D, :] = wk2
    c["wqk1"] = WQK1.astype(ml_dtypes.bfloat16)
    c["wk2"] = WK2.astype(ml_dtypes.bfloat16)

    # Wo lhsT bf16: rows 32h+1+j = Wo[:, 6h+j] (row 32h is the denominator)
    WOE = np.zeros((128, 32), np.float32)
    for h in range(H):
        for j in range(HD):
            WOE[32 * h + 1 + j, 0:D] = Wo[:, 6 * h + j]
    c["woe"] = WOE.astype(ml_dtypes.bfloat16)

    # LN selectors (fp32r): cb1 col 32p = -1/24 over band p; cb2 = +1/24
    CB1 = np.zeros((128, 128), np.float32)
    CB2 = np.zeros((128, 128), np.float32)
    for p in range(4):
        CB1[32 * p : 32 * p + D, 32 * p] = -1.0 / D
        CB2[32 * p : 32 * p + D, 32 * p] = 1.0 / D
    c["cb1"] = CB1
    c["cb2"] = CB2

    # broadcast selectors: col m -> 1 at row 32*(m//32); selg folds g
    SELR = np.zeros((128, 128), np.float32)
    SELG1 = np.zeros((128, 128), np.float32)
    SELG2 = np.zeros((128, 128), np.float32)
    for m in range(128):
        SELR[32 * (m // 32), m] = 1.0
        if m % 32 < D:
            SELG1[32 * (m // 32), m] = g1[m % 32]
            SELG2[32 * (m // 32), m] = g2[m % 32]
    c["selr"] = SELR
    c["selg1"] = SELG1
    c["selg2"] = SELG2
    c["selb"] = SELR.astype(ml_dtypes.bfloat16)

    # FFN W1 lhsT fp32r: variant p: rows 32p+d, col 64p+m = W1[m, d]
    W1E = np.zeros((128, 4 * 64), np.float32)
    for p in range(4):
        W1E[32 * p : 32 * p + D, 64 * p : 64 * p + DFF] = W1.T
    c["w1e"] = W1E.astype(ml_dtypes.bfloat16)

    # FFN W2 lhsT bf16: even variant rows 0:48, odd rows 64:112
    W2E = np.zeros((128, 2 * 32), np.float32)
    W2E[0:DFF, 0:D] = W2.T
    W2E[64 : 64 + DFF, 32 : 32 + D] = W2.T
    c["w2e"] = W2E.astype(ml_dtypes.bfloat16)

    c["idtb"] = np.eye(32, dtype=ml_dtypes.bfloat16)

    # banded biases (only used when nonzero)
    GB = np.zeros((128, 2), np.float32)
    for p in range(4):
        GB[32 * p : 32 * p + D, 0] = b1
        GB[32 * p : 32 * p + D, 1] = b2
    c["gb"] = GB
    return c


CONST_SPECS = {
    "wqk1": ((128, 128), BF16),
    "wk2": ((128, 128), BF16),
    "woe": ((128, 32), BF16),
    "cb1": ((128, 128), F32R),
    "cb2": ((128, 128), F32R),
    "selr": ((128, 128), F32R),
    "selg1": ((128, 128), F32R),
    "selg2": ((128, 128), F32R),
    "selb": ((128, 128), BF16),
    "w1e": ((128, 4 * 64), BF16),
    "w2e": ((128, 2 * 32), BF16),
    "idtb": ((32, 32), BF16),
    "gb": ((128, 2), F32),
}


def _pin_act_tables():
    """Pin Exp/Ln to natural_log_exp_and_others so LN's Ln+Exp never thrash."""
    import concourse.bacc as _bacc
    if getattr(_bacc, "_act_tables_pinned", False):
        return
    _orig = _bacc.get_activation_tables

    def _patched(arch):
        tables = dict(_orig(arch))
        keep = "natural_log_exp_and_others"
        for name in list(tables):
            if name != keep and (AF.Exp in tables[name] or AF.Ln in tables[name]):
                tables[name] = set()
        return tables

    _bacc.get_activation_tables = _patched
    _bacc._act_tables_pinned = True


def build_nc(nb: int = NB, use_b: bool = False) -> bass.Bass:
    _pin_act_tables()
    ngroups = nb // 4
    nc = bacc.Bacc()
    x_in = nc.dram_tensor("x", [nb, S, D], F32, kind="ExternalInput")
    out = nc.dram_tensor("out", [nb, S, D], F32, kind="ExternalOutput")
    cin = {
        k: nc.dram_tensor(k, list(sh), dt, kind="ExternalInput")
        for k, (sh, dt) in CONST_SPECS.items()
    }

    with tile.TileContext(nc) as tc:
        import contextlib

        ctx = contextlib.ExitStack()
        with ctx:
            constp = ctx.enter_context(tc.tile_pool(name="consts", bufs=1))
            vdp = ctx.enter_context(tc.tile_pool(name="vdp", bufs=1))
            xnp = ctx.enter_context(tc.tile_pool(name="xn", bufs=2))
            xtp = ctx.enter_context(tc.tile_pool(name="xt", bufs=2))
            xtbp = ctx.enter_context(tc.tile_pool(name="xtb", bufs=2))
            qkp = ctx.enter_context(tc.tile_pool(name="qk", bufs=2))
            ep = ctx.enter_context(tc.tile_pool(name="e", bufs=2))
            uosp = ctx.enter_context(tc.tile_pool(name="uos", bufs=2))
            rcpp = ctx.enter_context(tc.tile_pool(name="rcp", bufs=2))
            otp = ctx.enter_context(tc.tile_pool(name="ot", bufs=2))
            y1p = ctx.enter_context(tc.tile_pool(name="y1", bufs=2))
            lnp = ctx.enter_context(tc.tile_pool(name="ln", bufs=2))
            x1p = ctx.enter_context(tc.tile_pool(name="x1", bufs=2))
            hsp = ctx.enter_context(tc.tile_pool(name="hs", bufs=2))
            fsp = ctx.enter_context(tc.tile_pool(name="fs", bufs=2))
            ytp = ctx.enter_context(tc.tile_pool(name="yt", bufs=2))
            # PSUM: stt 2x[128,1024] + half 2x[128,512] + uo 2x[128,512] = 8 banks
            psA = ctx.enter_context(tc.tile_pool(name="psA", bufs=2, space="PSUM"))
            psB = ctx.enter_context(tc.tile_pool(name="psB", bufs=2, space="PSUM"))
            psC = ctx.enter_context(tc.tile_pool(name="psC", bufs=2, space="PSUM"))

            C = {}
            for k, (sh, dt) in CONST_SPECS.items():
                t = constp.tile(list(sh), dt, name=f"c_{k}")
                nc.sync.dma_start(out=t, in_=cin[k][:, :])
                C[k] = t
            eps_t = constp.tile([128, 1], F32, name="c_eps")
            nc.vector.memset(eps_t, EPS)

            # two persistent VD tiles [128, t4, hp2, i2, m128] fp8 (ping-pong).
            # ones preset at m = 32*(2hp+i) + {0, 7..31} inside each head band
            # (denominator column + finite padding rows for UO).
            vd_tiles = []
            for v in range(2):
                vt = vdp.tile([128, 4, 2, 2, 128], FP8, name=f"vd{v}")
                nc.gpsimd.memset(vt[:, :, :, :, :], 0.0)
                for hp in range(2):
                    for i in range(2):
                        m0 = 32 * (2 * hp + i)
                        nc.gpsimd.memset(vt[:, :, hp, i, m0 : m0 + 1], 1.0)
                        nc.gpsimd.memset(vt[:, :, hp, i, m0 + 7 : m0 + 32], 1.0)
                vd_tiles.append(vt)

            def ln_stages(Y, selg, bslice, OUT, outb=None):
                """LayerNorm over banded d of Y [128,512] fp32r, split into 4
                stages so PE never blocks on the DVE/ACT hops in between."""
                st = {}

                def s1():
                    mps = psB.tile([128, S], F32, name="mps", tag="half")
                    nc.tensor.matmul(mps[:, :], C["cb1"][:, :], Y[:, :],
                                     start=True, stop=True)
                    MU1 = lnp.tile([128, S], F32R, name="mu1", tag="mu")
                    nc.vector.tensor_copy(MU1[:, :], mps[:, :])
                    st["MU1"] = MU1

                def s2():
                    bcmu = psB.tile([128, S], F32, name="bcmu", tag="half")
                    nc.tensor.matmul(bcmu[:, :], C["selr"][:, :], st["MU1"][:, :],
                                     start=True, stop=True)
                    YC = lnp.tile([128, S], F32R, name="yc", tag="yc")
                    nc.vector.tensor_add(YC[:, :], Y[:, :], bcmu[:, :])
                    YC2 = lnp.tile([128, S], F32R, name="yc2", tag="yc2")
                    nc.gpsimd.tensor_mul(YC2[:, :], YC[:, :], YC[:, :])
                    st["YC"], st["YC2"] = YC, YC2

                def s3():
                    vps = psB.tile([128, S], F32, name="vps", tag="half")
                    nc.tensor.matmul(vps[:, :], C["cb2"][:, :], st["YC2"][:, :],
                                     start=True, stop=True)
                    LNV = lnp.tile([128, S], F32, name="lnv", tag="lnv")
                    nc.scalar.activation(LNV[:, :], vps[:, :], AF.Ln,
                                         bias=eps_t[:, :])
                    RS1 = lnp.tile([128, S], F32R, name="rs1", tag="rs")
                    nc.scalar.activation(RS1[:, :], LNV[:, :], AF.Exp, scale=-0.5)
                    st["RS1"] = RS1

                def s4():
                    bcrs = psB.tile([128, S], F32, name="bcrs", tag="half")
                    nc.tensor.matmul(bcrs[:, :], selg[:, :], st["RS1"][:, :],
                                     start=True, stop=True)
                    if use_b:
                        TMP = lnp.tile([128, S], F32R, name="lntmp", tag="tmp")
                        nc.vector.tensor_mul(TMP[:, :], st["YC"][:, :], bcrs[:, :])
                        nc.vector.tensor_scalar(out=OUT[:, :], in0=TMP[:, :],
                                                scalar1=bslice, scalar2=None,
                                                op0=ALU.add)
                    else:
                        nc.vector.tensor_mul(OUT[:, :], st["YC"][:, :], bcrs[:, :])
                    if outb is not None:
                        nc.gpsimd.tensor_copy(outb[:, :], OUT[:, :])

                return [s1, s2, s3, s4]

            def batch_head(g, p, XT4b):
                """QKV + scores + E-gen + AV for batch 4g+p. Returns state."""
                ps1 = psB.tile([128, S], F32, name="ps1", tag="half")
                nc.tensor.matmul(
                    ps1[:, :], C["wqk1"][32 * p : 32 * p + D, :],
                    XT4b[32 * p : 32 * p + D, :],
                    start=True, stop=True, tile_position=(32 * p, 0),
                )
                ps2 = psB.tile([128, S], F32, name="ps2", tag="half")
                nc.tensor.matmul(
                    ps2[:, :], C["wk2"][32 * p : 32 * p + D, :],
                    XT4b[32 * p : 32 * p + D, :],
                    start=True, stop=True, tile_position=(32 * p, 0),
                )
                QK = qkp.tile([128, 2 * S], BF16, name="qk")
                nc.vector.tensor_copy(QK[:, 0:S], ps1[:, :])
                nc.scalar.activation(QK[:, S : 2 * S], ps2[:, :], AF.Copy)

                # V^T via PE transposes of rows 0:32 (V lives at rows 8:32)
                psv = psB.tile([128, 4 * 32], BF16, name="psv", tag="half")
                for t in range(4):
                    nc.tensor.transpose(
                        psv[:, 32 * t : 32 * (t + 1)],
                        QK[0:32, 128 * t : 128 * (t + 1)],
                        C["idtb"][:, :],
                    )
                VD = vd_tiles[(4 * g + p) % 2]
                # VD[k, t, hp, i, 32*(2hp+i)+1 : +7] = psv[k, 32t+8+6(2hp+i) : +6]
                vd_dst = bass.AP(
                    tensor=VD.tensor, offset=VD.offset + 1,
                    ap=[list(VD.ap[0]), [512, 4], [320, 2], [160, 2], [1, 6]],
                )
                vd_src = bass.AP(
                    tensor=psv.tensor, offset=psv.offset + 8,
                    ap=[list(psv.ap[0]), [32, 4], [12, 2], [6, 2], [1, 6]],
                )
                nc.vector.tensor_copy(vd_dst, vd_src)

                # scores (S^T, raw) + E-gen (fp8e4m3 bits via affine)
                E = ep.tile([128, 4, 2, 2, S], FP8, name="e")
                for t in range(4):
                    for hp in range(2):
                        stt = psA.tile([128, 2 * S], F32, name="stt", tag="big")
                        for i in range(2):
                            h = 2 * hp + i
                            nc.tensor.matmul(
                                stt[:, 512 * i : 512 * (i + 1)],
                                QK[32 * h : 32 * h + HD,
                                   S + 128 * t : S + 128 * (t + 1)],
                                QK[32 * h : 32 * h + HD, 0:S],
                                start=True, stop=True,
                                tile_position=(32 * h, 0),
                                skip_group_check=True,
                            )
                        eng = EGEN_ENGINES[2 * t + hp]
                        e_out = E[:, t, hp, :, :].bitcast(U8)
                        s_in = stt.rearrange("p (i q) -> p i q", i=2)
                        if eng == "act":
                            nc.scalar.activation(e_out, s_in, AF.Copy,
                                                 bias=EB, scale=EA)
                        else:
                            nc.vector.tensor_scalar(
                                out=e_out, in0=s_in, scalar1=EA, scalar2=EB,
                                op0=ALU.mult, op1=ALU.add,
                            )

                return {"E": E, "VD": VD, "p": p}

            def batch_av(state):
                """fp8 DoubleRow AV, two heads per matmul via block-diag VD."""
                E, VD = state["E"], state["VD"]
                UO = psC.tile([128, S], F32, name="uo", tag="uo")
                idx = 0
                for t in range(4):
                    for hp in range(2):
                        nc.tensor.matmul(
                            UO[:, :], VD[:, t, hp, :, :], E[:, t, hp, :, :],
                            start=(idx == 0), stop=(idx == 7),
                            perf_mode=DR, tile_position=(0, 0),
                        )
                        idx += 1
                state["UO"] = UO

            def batch_tail(state, XT4, Y1):
                """normalize + Wo + residual for a batch (lagged emission)."""
                UO = state["UO"]
                p = state["p"]
                UOs = uosp.tile([128, S], BF16, name="uos")
                nc.scalar.activation(UOs[:, :], UO[:, :], AF.Copy)
                denps = psB.tile([128, S], F32, name="denps", tag="half")
                nc.tensor.matmul(denps[:, :], C["selb"][:, :], UOs[:, :],
                                 start=True, stop=True)
                RCP = rcpp.tile([128, S], F32, name="rcpt")
                nc.vector.reciprocal_approx_fast(RCP[:, :], denps[:, :])
                OTn = otp.tile([128, S], BF16, name="otn")
                nc.vector.tensor_mul(OTn[:, :], UOs[:, :], RCP[:, :])
                wops = psB.tile([32, S], F32, name="wops", tag="half")
                nc.tensor.matmul(wops[:, :], C["woe"][:, :], OTn[:, :],
                                 start=True, stop=True)
                nc.vector.tensor_add(
                    Y1[32 * p : 32 * p + 32, :], wops[:, :],
                    XT4[32 * p : 32 * p + 32, :],
                )

            def group_stages(g, Y1):
                """LN1 + FFN + LN2 + output store for group g as a stage list."""
                X1 = x1p.tile([128, S], F32R, name="x1")
                X1b = x1p.tile([128, S], BF16, name="x1b", tag="x1b")
                stages = ln_stages(Y1, C["selg1"], C["gb"][:, 0:1], X1, outb=X1b)
                st = {}

                def ffn_a():
                    F4 = psC.tile([128, S], F32, name="f4", tag="uo")
                    hps = psB.tile([128, S], F32, name="hps", tag="half")
                    for j in range(2):
                        nc.tensor.matmul(
                            hps[64 * j : 64 * j + 64, :],
                            C["w1e"][:, 64 * j : 64 * (j + 1)],
                            X1b[:, :],
                            start=True, stop=True, tile_position=(0, 64 * j),
                            skip_group_check=True,
                        )
                    HS = hsp.tile([128, S], BF16, name="hs")
                    nc.scalar.activation(HS[:, :], hps[:, :], AF.Relu)
                    st["F4"], st["HS"] = F4, HS

                def ffn_b():
                    F4 = st["F4"]
                    for j in range(2):
                        nc.tensor.matmul(
                            F4[32 * j : 32 * j + 32, :],
                            C["w2e"][:, 32 * j : 32 * (j + 1)],
                            st["HS"][:, :],
                            start=True, stop=True, tile_position=(0, 32 * j),
                            skip_group_check=True,
                        )
                    hps = psB.tile([128, S], F32, name="hps2", tag="half")
                    for j in range(2):
                        nc.tensor.matmul(
                            hps[64 * j : 64 * j + 64, :],
                            C["w1e"][:, 64 * (2 + j) : 64 * (3 + j)],
                            X1b[:, :],
                            start=True, stop=True, tile_position=(0, 64 * j),
                            skip_group_check=True,
                        )
                    HS2 = hsp.tile([128, S], BF16, name="hs2", tag="hs2")
                    nc.scalar.activation(HS2[:, :], hps[:, :], AF.Relu)
                    st["HS2"] = HS2

                def ffn_c():
                    F4 = st["F4"]
                    for j in range(2):
                        nc.tensor.matmul(
                            F4[32 * (2 + j) : 32 * (3 + j), :],
                            C["w2e"][:, 32 * j : 32 * (j + 1)],
                            st["HS2"][:, :],
                            start=True, stop=True, tile_position=(0, 32 * (2 + j)),
                            skip_group_check=True,
                        )
                    FS0 = fsp.tile([128, S], F32R, name="fs0", tag="fs0")
                    nc.scalar.activation(FS0[:, :], F4[:, :], AF.Relu)
                    FS = fsp.tile([128, S], F32R, name="fst", tag="fst")
                    nc.gpsimd.tensor_add(FS[:, :], FS0[:, :], X1[:, :])
                    st["FS"] = FS

                stages += [ffn_a, ffn_b, ffn_c]

                Y2N = ytp.tile([128, S], F32, name="y2n", tag="y2n")
                ln2 = [None]

                def ln2_s1():
                    ln2[0] = ln_stages(st["FS"], C["selg2"], C["gb"][:, 1:2], Y2N)
                    ln2[0][0]()

                def emit_out():
                    Y2T = ytp.tile([128, S], F32, name="y2t", tag="y2t")
                    nc.vector.transpose(Y2T[:, :], Y2N[:, :])
                    for p in range(4):
                        b = 4 * g + p
                        nc.sync.dma_start(
                            out=out[b].rearrange("(f r) d -> r f d", r=32),
                            in_=Y2T[32 * p : 32 * p + 32, :].rearrange(
                                "r (f c) -> r f c", c=32
                            )[:, :, 0:D],
                        )

                stages += [ln2_s1,
                           lambda: ln2[0][1](),
                           lambda: ln2[0][2](),
                           lambda: ln2[0][3](),
                           emit_out]
                return stages

            # slot machine: headA(i) | AV(i-1) | tail(i-2) | <=3 group stages
            states = {}
            gctx = {}
            pending = []
            nslots = 4 * ngroups

            def emit_slot(i):
                g, p = divmod(i, 4)
                if p == 0:
                    U = xnp.tile([128, 16, 32], F32, name="xu")
                    nc.gpsimd.memset(U[:, :, D:32], 0.0)
                    for pp in range(4):
                        nc.sync.dma_start(
                            out=U[32 * pp : 32 * pp + 32, :, 0:D],
                            in_=x_in[4 * g + pp].rearrange("(f c) d -> c f d", c=32),
                        )
                    XT4 = xtp.tile([128, S], F32, name="xt4")
                    nc.vector.transpose(XT4[:, :], U.rearrange("p a b -> p (a b)"))
                    XT4b = xtbp.tile([128, S], BF16, name="xt4b")
                    nc.gpsimd.tensor_copy(XT4b[:, :], XT4[:, :])
                    Y1 = y1p.tile([128, S], F32R, name="y1")
                    gctx[g] = (XT4, XT4b, Y1)
                XT4, XT4b, Y1 = gctx[g]
                states[i] = batch_head(g, p, XT4b)
                if i - 1 in states:
                    batch_av(states[i - 1])
                if i - 2 in states:
                    s = states.pop(i - 2)
                    g2 = (i - 2) // 4
                    batch_tail(s, gctx[g2][0], gctx[g2][2])
                    if (i - 2) % 4 == 3:
                        pending.extend(group_stages(g2, gctx[g2][2]))
                for _ in range(3):
                    if pending:
                        pending.pop(0)()

            for i in range(nslots):
                emit_slot(i)
            # drain
            batch_av(states[nslots - 1])
            for i in (nslots - 2, nslots - 1):
                s = states.pop(i)
                g2 = i // 4
                batch_tail(s, gctx[g2][0], gctx[g2][2])
                if i % 4 == 3:
                    pending.extend(group_stages(g2, gctx[g2][2]))
            while pending:
                pending.pop(0)()

    nc.compile()
    return nc


_NC_CACHE: dict[tuple, bass.Bass] = {}


def _get_nc(nb: int, use_b: bool = False) -> bass.Bass:
    key = (nb, use_b)
    if key not in _NC_CACHE:
        _NC_CACHE[key] = build_nc(nb, use_b)
    return _NC_CACHE[key]


def kernel(x, Wq, Wk, Wv, Wo, W1, W2, g1, b1, g2, b2):
    x = np.asarray(x, np.float32)
    args = [np.asarray(a, np.float32) for a in (Wq, Wk, Wv, Wo, W1, W2, g1, b1, g2, b2)]
    consts = _host_consts(*args)
    use_b = bool(np.any(args[7]) or np.any(args[9]))
    nc = _get_nc(NB, use_b)
    in_maps = []
    for c in range(NCORES):
        m = {"x": np.ascontiguousarray(x[c * NB : (c + 1) * NB])}
        m.update(consts)
        in_maps.append(m)
    res = run_bass_kernel_spmd(nc, in_maps, list(range(NCORES)))
    return np.concatenate([r["out"] for r in res.results], axis=0)
